# revision 1
# baseline (speedup 1.0000x reference)
"""Bootstrapped BCE-with-logits loss (top-25% hard-pixel mining) on 8 TRN2 cores.

Math: loss_pixel = softplus(x) - x*y  (== max(x,0) - x*y + log1p(exp(-|x|)))
For each row, mean of top-k (k = N/4) pixel losses, then global mean.

Key identity: with t = k-th largest value of row v,
    sum_topk(row) = k*t + sum_j relu(v_j - t)
and the RHS is stationary in t at t* (d/dt = k - count(v > t) = 0), so an
approximate per-row threshold gives only O(rho*N*delta^2) error.  The kernel
estimates t per row with Newton iterations on subsample counts (on-device),
then does one fused relu+accumulate pass.

Sharding: data-parallel over the batch dim: core c handles rows 8c..8c+7.
Each core's 8 rows are laid out as SBUF [128 partitions x 16384], partition
p holding elements of row p//16.  Inputs are cast to bf16 on the host
(halves DMA traffic; validated rel-err ~3e-5 vs f32 reference).
Per-core output is [128, 2] f32: col0 = per-partition sum of relu(v - t),
col1 = per-partition (replicated per row) threshold t.  The host combines:
total = sum(col0) + k * sum_rows(t_row); answer = total / (B*k).
"""

import numpy as np
import ml_dtypes

_NCORES = 8
_B = 64
_HW = 512 * 512            # 262144 pixels per row
_RPC = _B // _NCORES       # 8 rows per core
_P = 128                   # SBUF partitions
_FREE = _RPC * _HW // _P   # 16384 elements per partition
# tapered chunk widths: small first chunk lets ACT start early, big middle
# chunks amortize per-op overhead, small last chunk shrinks the serial tail
# (DMA->exp->ln->sub->fin)
_CHUNK_W = [1024, 2048, 2048, 2048, 4096, 4096, 1024]
_CHUNK_OFF = [sum(_CHUNK_W[:i]) for i in range(len(_CHUNK_W))]
_NCHUNK = len(_CHUNK_W)
assert sum(_CHUNK_W) == _FREE
_K = _HW // 4              # 65536 (top-k per row)
_PPR = _P // _RPC          # 16 partitions per row

# Newton refinement schedule: (chunk index used for counting, sample width
# in columns, half-width h, clamp on the update).  t0 is a
# distribution-informed initial guess; the rounds make the kernel robust to
# shifts in the loss distribution.
_T0 = 0.92
_ROUNDS = [(1, 2048, 0.12, 0.50)]

_BF16 = ml_dtypes.bfloat16

# True: use the native ACT Softplus LUT (1 pass).  False: exp then ln(1+e)
# (2 passes; both functions live in the natural_log_exp_and_others table set).
_USE_SOFTPLUS = False
# per-chunk engine assignment for load balancing:
#   mul (x*y):            'v' = vector, 'g' = gpsimd
#   sub (sp - p -> V):    'v' | 'g'
#   final relu+accum:     'v' | 'g' | 'a' (scalar engine Relu w/ bias)
_MUL_ENG = "vvvvvvvv"
_SUB_ENG = "vvvvvvvv"
_FIN_ENG = "vvvvvvvv"
# debug: which pipeline stages to emit (for stall localization probes)
_STAGES = frozenset({"act", "mulsub", "newton", "final"})
# store exp(x) intermediate in bf16 (both ACT passes then 16-bit)
_E_BF16 = False
# input-chunk double-buffer depth
_IO_BUFS = 3
# softplus-output buffer depth: sp_j is produced by ACT and consumed by the
# vector engine's sub_j; a deeper pool stops ACT stalling on DVE's lag
_SP_BUFS = 4

_cached_nc = None


def build_bass(reps=1):
    """Build the (SPMD, per-core identical) Bass program.

    reps > 1 repeats the whole body serially inside one NEFF — used to
    measure per-iteration device time without per-launch host overhead.
    """
    from concourse import bacc, mybir
    from concourse.tile import TileContext

    dt = mybir.dt
    Act = mybir.ActivationFunctionType
    Alu = mybir.AluOpType
    _E_DT = dt.bfloat16 if _E_BF16 else dt.float32

    nc = bacc.Bacc("TRN2", target_bir_lowering=False, debug=False)

    # x and y interleaved per chunk ([..., j, 0, :] = x-chunk j, [..., 1, :] = y)
    # so each chunk needs exactly ONE dma_start -> consumers carry a single
    # sync-wait (the TT ISA slot only fits one).
    xy_ext = nc.declare_dram_parameter(
        "xy", [_P, 2 * _FREE], dt.bfloat16, isOutput=False
    )
    out_ext = nc.declare_dram_parameter("out", [_P, 2], dt.float32, isOutput=True)

    with TileContext(nc) as tc:
        with (
            tc.tile_pool(name="io", bufs=_IO_BUFS) as io_pool,
            tc.tile_pool(name="tmp", bufs=2) as tmp_pool,
            tc.tile_pool(name="spp", bufs=_SP_BUFS) as sp_pool,
            tc.tile_pool(name="persist", bufs=1) as persist,
            tc.tile_pool(name="small", bufs=1) as small,
            tc.tile_pool(name="psum", bufs=2, space="PSUM") as psum_pool,
        ):
            # persistent loss tile: all 8 rows of this core
            V = persist.tile([_P, _FREE], dt.bfloat16)

            # constants: row-indicator matrices for cross-partition
            # (per-row) reductions/broadcasts via the tensor engine.
            # Built on-device with iota (no DMA issues on the critical path):
            # ind8[p, b] = (p//16 == b), ind8T[b, p] = (p//16 == b)
            ind8 = small.tile([_P, _RPC], dt.float32)     # [128, 8]
            ind8T = small.tile([_RPC, _P], dt.float32)    # [8, 128]
            rid = small.tile([_P, 1], dt.int32)
            nc.gpsimd.iota(rid[:], [[0, 1]], channel_multiplier=1)
            nc.vector.tensor_scalar(
                rid[:], rid[:], 4, None, Alu.logical_shift_right
            )
            rid_f = small.tile([_P, 1], dt.float32)
            nc.vector.tensor_copy(rid_f[:], rid[:])
            col8 = small.tile([_P, _RPC], dt.int32)
            nc.gpsimd.iota(col8[:], [[1, _RPC]], channel_multiplier=0)
            col8_f = small.tile([_P, _RPC], dt.float32)
            nc.vector.tensor_copy(col8_f[:], col8[:])
            nc.vector.tensor_scalar(
                ind8[:], col8_f[:], rid_f[:], None, Alu.is_equal
            )
            colP = small.tile([_RPC, _P], dt.int32)
            nc.gpsimd.iota(colP[:], [[1, _P]], channel_multiplier=0)
            nc.vector.tensor_scalar(
                colP[:], colP[:], 4, None, Alu.logical_shift_right
            )
            rid8 = small.tile([_RPC, 1], dt.int32)
            nc.gpsimd.iota(rid8[:], [[0, 1]], channel_multiplier=1)
            rid8_f = small.tile([_RPC, 1], dt.float32)
            nc.vector.tensor_copy(rid8_f[:], rid8[:])
            colP_f = small.tile([_RPC, _P], dt.float32)
            nc.vector.tensor_copy(colP_f[:], colP[:])
            nc.vector.tensor_scalar(
                ind8T[:], colP_f[:], rid8_f[:], None, Alu.is_equal
            )

            # current per-row threshold, broadcast across the row's partitions
            t_bc = small.tile([_P, 1], dt.float32)
            t8 = small.tile([_RPC, 1], dt.float32)

            # issue chunk DMAs from different engine sequencers so the
            # per-dma descriptor-generation (~1.2us) doesn't serialize on
            # one engine and delay early transfers
            dma_issuers = [nc.sync] * 8

            def produce_chunk(j):
                w, off = _CHUNK_W[j], _CHUNK_OFF[j]
                xyt = io_pool.tile([_P, 2 * w], dt.bfloat16, tag="xyt")
                dma_issuers[j].dma_start(
                    xyt[:], xy_ext[:, 2 * off:2 * off + 2 * w]
                )
                xt = xyt[:, 0:w]
                yt = xyt[:, w:2 * w]
                sp = sp_pool.tile([_P, w], dt.bfloat16, tag="sp")
                if "act" not in _STAGES:
                    nc.vector.tensor_copy(sp[:], xt)
                elif _USE_SOFTPLUS:
                    nc.scalar.activation(sp[:], xt, Act.Softplus)
                else:
                    e = tmp_pool.tile([_P, w], _E_DT, tag="e")
                    nc.scalar.activation(e[:], xt, Act.Exp)
                    nc.scalar.activation(sp[:], e[:], Act.Ln, bias=1.0)
                if "mulsub" not in _STAGES:
                    nc.vector.tensor_copy(V[:, off:off + w], sp[:])
                    return
                p = tmp_pool.tile([_P, w], dt.bfloat16, tag="p")
                mul_eng = nc.gpsimd if _MUL_ENG[j] == "g" else nc.vector
                sub_eng = nc.gpsimd if _SUB_ENG[j] == "g" else nc.vector
                mul_eng.tensor_tensor(p[:], xt, yt, Alu.mult)
                sub_eng.tensor_tensor(
                    V[:, off:off + w], sp[:], p[:], Alu.subtract
                )

            def newton_round(ridx, chunk, width, h, clamp):
                vc = V[:, _CHUNK_OFF[chunk]:_CHUNK_OFF[chunk] + width]
                n_samp = width * _PPR  # per-row sample count (over 16 parts)
                cnt = small.tile([_P, 3], dt.float32, tag=f"cnt{ridx}")
                msk = tmp_pool.tile([_P, width], dt.bfloat16, tag=f"msk{ridx}")
                # Counting on the scalar engine (Sign is in the same table
                # set as Exp/Ln): accum = sum sign(v - thr) = 2*count - n.
                # Only round 0 exists, so thresholds are compile-time floats.
                # DVE is_ge+accum runs 4x and keeps the scalar-engine stream
                # (the pipeline spine) free; S = count here, n kept for the
                # shared S-space Newton formula below via S = 2c - n.
                assert ridx == 0
                for i, off in enumerate((-h, 0.0, h)):
                    nc.vector.tensor_scalar(
                        msk[:], vc, float(_T0 + off), None, Alu.is_ge,
                        Alu.add, accum_out=cnt[:, i:i + 1],
                    )
                # per-row S: [8, 3] = ind8.T @ cnt
                pc = psum_pool.tile([_RPC, 3], dt.float32, tag="pc")
                nc.tensor.matmul(pc[:], ind8[:], cnt[:])
                rc = small.tile([_RPC, 3], dt.float32, tag=f"rc{ridx}")
                nc.vector.tensor_copy(rc[:], pc[:])
                # Newton update: t += clamp(2h*(c_mid - n/4)/(c_lo - c_hi))
                num = small.tile([_RPC, 1], dt.float32, tag=f"num{ridx}")
                den = small.tile([_RPC, 1], dt.float32, tag=f"den{ridx}")
                q = small.tile([_RPC, 1], dt.float32, tag=f"q{ridx}")
                nc.vector.tensor_scalar(
                    num[:], rc[:, 1:2], float(n_samp / 4), float(2.0 * h),
                    Alu.subtract, Alu.mult,
                )
                nc.vector.tensor_tensor(den[:], rc[:, 0:1], rc[:, 2:3], Alu.subtract)
                rden = small.tile([_RPC, 1], dt.float32, tag=f"rden{ridx}")
                nc.vector.reciprocal(rden[:], den[:])
                nc.vector.tensor_tensor(q[:], num[:], rden[:], Alu.mult)
                nc.vector.tensor_scalar(
                    q[:], q[:], float(clamp), float(-clamp), Alu.min, Alu.max
                )
                nc.vector.tensor_tensor(t8[:], t8[:], q[:], Alu.add)
                # broadcast t8 [8,1] -> t_bc [128,1]
                pt = psum_pool.tile([_P, 1], dt.float32, tag="pt")
                nc.tensor.matmul(pt[:], ind8T[:], t8[:])
                nc.vector.tensor_copy(t_bc[:], pt[:])

            # final pass helper: acc[:, j] = sum_chunk relu(v - t).
            # tensor_scalar runs 4x (both read ports + bf16 packing); the
            # 2-src scalar_tensor_tensor variant would be 1x, so two ts ops
            # beat one stt by ~2x.
            acc = small.tile([_P, _NCHUNK], dt.float32)
            neg_t = small.tile([_P, 1], dt.float32)

            def final_chunk(j):
                if "final" not in _STAGES:
                    return
                w, off = _CHUNK_W[j], _CHUNK_OFF[j]
                rl = tmp_pool.tile([_P, w], dt.bfloat16, tag="rl")
                vj = V[:, off:off + w]
                if _FIN_ENG[j] == "a":
                    nc.scalar.activation(
                        rl[:], vj, Act.Relu,
                        bias=neg_t[:], accum_out=acc[:, j:j + 1],
                    )
                else:
                    eng = nc.gpsimd if _FIN_ENG[j] == "g" else nc.vector
                    # rl = max(v - t, 0)
                    eng.tensor_scalar(
                        rl[:], vj, t_bc[:], 0.0, Alu.subtract, Alu.max
                    )
                    # accum_out = sum(rl)   (op0 max(.,0) is a no-op on rl)
                    eng.tensor_scalar(
                        rl[:], rl[:], 0.0, None, Alu.max, Alu.add,
                        accum_out=acc[:, j:j + 1],
                    )

            # --- main pipeline ---
            # chunks 0/1 first, then the Newton round (counts on chunk 1),
            # then finals interleaved with remaining production so the only
            # work after the last ln is one small sub + final.
            # reps > 1 serially repeats the body (device-time measurement).
            round_chunks = max(r[0] for r in _ROUNDS) + 1 if _ROUNDS else 0
            for _rep in range(reps):
                nc.vector.memset(t_bc[:], _T0)
                nc.vector.memset(t8[:], _T0)
                nc.vector.memset(acc[:], 0.0)
                for j in range(round_chunks):
                    produce_chunk(j)
                if "newton" in _STAGES:
                    for ridx, (chunk, width, h, clamp) in enumerate(_ROUNDS):
                        newton_round(ridx, chunk, width, h, clamp)
                nc.vector.tensor_scalar_mul(neg_t[:], t_bc[:], -1.0)
                for j in range(round_chunks):
                    final_chunk(j)
                for j in range(round_chunks, _NCHUNK):
                    produce_chunk(j)
                    final_chunk(j)

                out_t = small.tile([_P, 2], dt.float32)
                nc.vector.tensor_reduce(
                    out_t[:, 0:1], acc[:], mybir.AxisListType.X, Alu.add
                )
                nc.vector.tensor_copy(out_t[:, 1:2], t_bc[:])
                nc.sync.dma_start(out_ext[:], out_t[:])

    # Steer the ACT table chooser to the one set holding BOTH Exp and Ln so
    # it loads a single table instead of thrashing exp<->ln sets per chunk.
    # Only the chooser's view is filtered; list order (= act_func_set_id)
    # is preserved, and the runtime set genuinely contains both functions.
    from concourse import bacc as _bacc_mod
    _orig_tables = _bacc_mod.get_activation_tables

    def _steered_tables(arch):
        tabs = dict(_orig_tables(arch))
        # pin every function this kernel uses to the one set with Exp+Ln
        used = {Act.Exp, Act.Ln, Act.Sign, Act.Relu}
        combined = [n for n, fns in tabs.items() if {Act.Exp, Act.Ln} <= fns]
        if combined:
            keep = combined[0]
            tabs = {
                n: (fns if n == keep else (fns - used))
                for n, fns in tabs.items()
            }
        return tabs

    _bacc_mod.get_activation_tables = _steered_tables
    try:
        nc.compile()
    finally:
        _bacc_mod.get_activation_tables = _orig_tables
    return nc


def _shard_inputs(pred_logits, gts):
    x = np.ascontiguousarray(pred_logits, dtype=np.float32).reshape(_B, _HW)
    y = np.ascontiguousarray(gts, dtype=np.float32).reshape(_B, _HW)
    xb = x.astype(_BF16)
    yb = y.astype(_BF16)
    in_maps = []
    for c in range(_NCORES):
        sl = slice(c * _RPC, (c + 1) * _RPC)
        xs = xb[sl].reshape(_P, _FREE)
        ys = yb[sl].reshape(_P, _FREE)
        # interleave x/y per (variable-width) chunk: [... x_w | y_w ...]
        blocks = []
        for w, off in zip(_CHUNK_W, _CHUNK_OFF):
            blocks.append(xs[:, off:off + w])
            blocks.append(ys[:, off:off + w])
        xy = np.concatenate(blocks, axis=1)
        in_maps.append({"xy": np.ascontiguousarray(xy)})
    return in_maps


def _combine(results):
    total = 0.0
    for c in range(_NCORES):
        out = np.asarray(results[c]["out"], dtype=np.float64)  # [128, 2]
        total += out[:, 0].sum()
        total += _K * out[::_PPR, 1].sum()
    return np.float32(total / (_B * _K))


def kernel(pred_logits, gts):
    from concourse.bass_utils import run_bass_kernel_spmd

    global _cached_nc
    if _cached_nc is None:
        _cached_nc = build_bass()
    in_maps = _shard_inputs(pred_logits, gts)
    res = run_bass_kernel_spmd(_cached_nc, in_maps, list(range(_NCORES)))
    return _combine(res.results)



# revision 4
# speedup vs baseline: 49.1960x; 49.1960x over previous
"""Bootstrapped BCE-with-logits loss (top-25% hard-pixel mining) on 8 TRN2 cores.

Math: loss_pixel = softplus(x) - x*y  (== max(x,0) - x*y + log1p(exp(-|x|)))
For each row, mean of top-k (k = N/4) pixel losses, then global mean.

Key identity: with t = k-th largest value of row v,
    sum_topk(row) = k*t + sum_j relu(v_j - t)
and the RHS is stationary in t at t* (d/dt = k - count(v > t) = 0), so an
approximate per-row threshold gives only O(rho*N*delta^2) error.  The kernel
estimates t per row with Newton iterations on subsample counts (on-device),
then does one fused relu+accumulate pass.

Sharding: data-parallel over the batch dim: core c handles rows 8c..8c+7.
Each core's 8 rows are laid out as SBUF [128 partitions x 16384], partition
p holding elements of row p//16.  Inputs are cast to bf16 on the host
(halves DMA traffic; validated rel-err ~3e-5 vs f32 reference).
Per-core output is [128, 2] f32: col0 = per-partition sum of relu(v - t),
col1 = per-partition (replicated per row) threshold t.  The host combines:
total = sum(col0) + k * sum_rows(t_row); answer = total / (B*k).
"""

import numpy as np
import ml_dtypes

_NCORES = 8
_B = 64
_HW = 512 * 512            # 262144 pixels per row
_RPC = _B // _NCORES       # 8 rows per core
_P = 128                   # SBUF partitions
_FREE = _RPC * _HW // _P   # 16384 elements per partition
# tapered chunk widths: small first chunk lets ACT start early, big middle
# chunks amortize per-op overhead, small last chunk shrinks the serial tail
# (DMA->exp->ln->sub->fin)
_CHUNK_W = [1024, 2048, 2048, 2048, 4096, 4096, 1024]
_CHUNK_OFF = [sum(_CHUNK_W[:i]) for i in range(len(_CHUNK_W))]
_NCHUNK = len(_CHUNK_W)
assert sum(_CHUNK_W) == _FREE
_K = _HW // 4              # 65536 (top-k per row)
_PPR = _P // _RPC          # 16 partitions per row

# Newton refinement schedule: (chunk index used for counting, sample width
# in columns, half-width h, clamp on the update).  t0 is a
# distribution-informed initial guess; the rounds make the kernel robust to
# shifts in the loss distribution.
_T0 = 0.92
_ROUNDS = [(1, 2048, 0.12, 0.50)]

_BF16 = ml_dtypes.bfloat16

# True: use the native ACT Softplus LUT (1 pass).  False: exp then ln(1+e)
# (2 passes; both functions live in the natural_log_exp_and_others table set).
_USE_SOFTPLUS = False
# per-chunk engine assignment for load balancing:
#   mul (x*y):            'v' = vector, 'g' = gpsimd
#   sub (sp - p -> V):    'v' | 'g'
#   final relu+accum:     'v' | 'g' | 'a' (scalar engine Relu w/ bias)
_MUL_ENG = "vvvvvvvv"
_SUB_ENG = "vvvvvvvv"
_FIN_ENG = "vvvvvvvv"
# debug: which pipeline stages to emit (for stall localization probes)
_STAGES = frozenset({"act", "mulsub", "newton", "final"})
# store exp(x) intermediate in bf16 (both ACT passes then 16-bit)
_E_BF16 = False
# input-chunk double-buffer depth
_IO_BUFS = 3
# softplus-output buffer depth: sp_j is produced by ACT and consumed by the
# vector engine's sub_j; a deeper pool stops ACT stalling on DVE's lag
_SP_BUFS = 4

_cached_nc = None


def build_bass(reps=1, loop=None):
    """Build the (SPMD, per-core identical) Bass program.

    reps > 1 repeats the whole body serially inside one NEFF — used to
    measure per-iteration device time without per-launch host overhead.
    loop = N additionally wraps the reps-unrolled body in a tc.For_i
    hardware loop of N iterations (constant NEFF size, huge rep counts).
    """
    from concourse import bacc, mybir
    from concourse.tile import TileContext

    dt = mybir.dt
    Act = mybir.ActivationFunctionType
    Alu = mybir.AluOpType
    _E_DT = dt.bfloat16 if _E_BF16 else dt.float32

    nc = bacc.Bacc("TRN2", target_bir_lowering=False, debug=False)

    # x and y interleaved per chunk ([..., j, 0, :] = x-chunk j, [..., 1, :] = y)
    # so each chunk needs exactly ONE dma_start -> consumers carry a single
    # sync-wait (the TT ISA slot only fits one).
    xy_ext = nc.declare_dram_parameter(
        "xy", [_P, 2 * _FREE], dt.bfloat16, isOutput=False
    )
    out_ext = nc.declare_dram_parameter("out", [_P, 2], dt.float32, isOutput=True)

    with TileContext(nc) as tc:
        with (
            tc.tile_pool(name="io", bufs=_IO_BUFS) as io_pool,
            tc.tile_pool(name="tmp", bufs=2) as tmp_pool,
            tc.tile_pool(name="spp", bufs=_SP_BUFS) as sp_pool,
            tc.tile_pool(name="persist", bufs=1) as persist,
            tc.tile_pool(name="small", bufs=1) as small,
            tc.tile_pool(name="psum", bufs=2, space="PSUM") as psum_pool,
        ):
            # persistent loss tile: all 8 rows of this core
            V = persist.tile([_P, _FREE], dt.bfloat16)

            # constants: row-indicator matrices for cross-partition
            # (per-row) reductions/broadcasts via the tensor engine.
            # Built on-device with iota (no DMA issues on the critical path):
            # ind8[p, b] = (p//16 == b), ind8T[b, p] = (p//16 == b)
            ind8 = small.tile([_P, _RPC], dt.float32)     # [128, 8]
            ind8T = small.tile([_RPC, _P], dt.float32)    # [8, 128]
            rid = small.tile([_P, 1], dt.int32)
            nc.gpsimd.iota(rid[:], [[0, 1]], channel_multiplier=1)
            nc.vector.tensor_scalar(
                rid[:], rid[:], 4, None, Alu.logical_shift_right
            )
            rid_f = small.tile([_P, 1], dt.float32)
            nc.vector.tensor_copy(rid_f[:], rid[:])
            col8 = small.tile([_P, _RPC], dt.int32)
            nc.gpsimd.iota(col8[:], [[1, _RPC]], channel_multiplier=0)
            col8_f = small.tile([_P, _RPC], dt.float32)
            nc.vector.tensor_copy(col8_f[:], col8[:])
            nc.vector.tensor_scalar(
                ind8[:], col8_f[:], rid_f[:], None, Alu.is_equal
            )
            colP = small.tile([_RPC, _P], dt.int32)
            nc.gpsimd.iota(colP[:], [[1, _P]], channel_multiplier=0)
            nc.vector.tensor_scalar(
                colP[:], colP[:], 4, None, Alu.logical_shift_right
            )
            rid8 = small.tile([_RPC, 1], dt.int32)
            nc.gpsimd.iota(rid8[:], [[0, 1]], channel_multiplier=1)
            rid8_f = small.tile([_RPC, 1], dt.float32)
            nc.vector.tensor_copy(rid8_f[:], rid8[:])
            colP_f = small.tile([_RPC, _P], dt.float32)
            nc.vector.tensor_copy(colP_f[:], colP[:])
            nc.vector.tensor_scalar(
                ind8T[:], colP_f[:], rid8_f[:], None, Alu.is_equal
            )

            # current per-row threshold, broadcast across the row's partitions
            t_bc = small.tile([_P, 1], dt.float32)
            t8 = small.tile([_RPC, 1], dt.float32)

            # issue chunk DMAs from different engine sequencers so the
            # per-dma descriptor-generation (~1.2us) doesn't serialize on
            # one engine and delay early transfers
            dma_issuers = [nc.sync] * 8

            def produce_chunk(j):
                w, off = _CHUNK_W[j], _CHUNK_OFF[j]
                xyt = io_pool.tile([_P, 2 * w], dt.bfloat16, tag="xyt")
                dma_issuers[j].dma_start(
                    xyt[:], xy_ext[:, 2 * off:2 * off + 2 * w]
                )
                xt = xyt[:, 0:w]
                yt = xyt[:, w:2 * w]
                sp = sp_pool.tile([_P, w], dt.bfloat16, tag="sp")
                if "act" not in _STAGES:
                    nc.vector.tensor_copy(sp[:], xt)
                elif _USE_SOFTPLUS:
                    nc.scalar.activation(sp[:], xt, Act.Softplus)
                else:
                    e = tmp_pool.tile([_P, w], _E_DT, tag="e")
                    nc.scalar.activation(e[:], xt, Act.Exp)
                    nc.scalar.activation(sp[:], e[:], Act.Ln, bias=1.0)
                if "mulsub" not in _STAGES:
                    nc.vector.tensor_copy(V[:, off:off + w], sp[:])
                    return
                p = tmp_pool.tile([_P, w], dt.bfloat16, tag="p")
                mul_eng = nc.gpsimd if _MUL_ENG[j] == "g" else nc.vector
                sub_eng = nc.gpsimd if _SUB_ENG[j] == "g" else nc.vector
                mul_eng.tensor_tensor(p[:], xt, yt, Alu.mult)
                sub_eng.tensor_tensor(
                    V[:, off:off + w], sp[:], p[:], Alu.subtract
                )

            def newton_round(ridx, chunk, width, h, clamp):
                vc = V[:, _CHUNK_OFF[chunk]:_CHUNK_OFF[chunk] + width]
                n_samp = width * _PPR  # per-row sample count (over 16 parts)
                cnt = small.tile([_P, 3], dt.float32, tag=f"cnt{ridx}")
                msk = tmp_pool.tile([_P, width], dt.bfloat16, tag=f"msk{ridx}")
                # Counting on the scalar engine (Sign is in the same table
                # set as Exp/Ln): accum = sum sign(v - thr) = 2*count - n.
                # Only round 0 exists, so thresholds are compile-time floats.
                # DVE is_ge+accum runs 4x and keeps the scalar-engine stream
                # (the pipeline spine) free; S = count here, n kept for the
                # shared S-space Newton formula below via S = 2c - n.
                assert ridx == 0
                for i, off in enumerate((-h, 0.0, h)):
                    nc.vector.tensor_scalar(
                        msk[:], vc, float(_T0 + off), None, Alu.is_ge,
                        Alu.add, accum_out=cnt[:, i:i + 1],
                    )
                # per-row S: [8, 3] = ind8.T @ cnt
                pc = psum_pool.tile([_RPC, 3], dt.float32, tag="pc")
                nc.tensor.matmul(pc[:], ind8[:], cnt[:])
                rc = small.tile([_RPC, 3], dt.float32, tag=f"rc{ridx}")
                nc.vector.tensor_copy(rc[:], pc[:])
                # Newton update: t += clamp(2h*(c_mid - n/4)/(c_lo - c_hi))
                num = small.tile([_RPC, 1], dt.float32, tag=f"num{ridx}")
                den = small.tile([_RPC, 1], dt.float32, tag=f"den{ridx}")
                q = small.tile([_RPC, 1], dt.float32, tag=f"q{ridx}")
                nc.vector.tensor_scalar(
                    num[:], rc[:, 1:2], float(n_samp / 4), float(2.0 * h),
                    Alu.subtract, Alu.mult,
                )
                nc.vector.tensor_tensor(den[:], rc[:, 0:1], rc[:, 2:3], Alu.subtract)
                rden = small.tile([_RPC, 1], dt.float32, tag=f"rden{ridx}")
                nc.vector.reciprocal(rden[:], den[:])
                nc.vector.tensor_tensor(q[:], num[:], rden[:], Alu.mult)
                nc.vector.tensor_scalar(
                    q[:], q[:], float(clamp), float(-clamp), Alu.min, Alu.max
                )
                nc.vector.tensor_tensor(t8[:], t8[:], q[:], Alu.add)
                # broadcast t8 [8,1] -> t_bc [128,1]
                pt = psum_pool.tile([_P, 1], dt.float32, tag="pt")
                nc.tensor.matmul(pt[:], ind8T[:], t8[:])
                nc.vector.tensor_copy(t_bc[:], pt[:])

            # final pass helper: acc[:, j] = sum_chunk relu(v - t).
            # tensor_scalar runs 4x (both read ports + bf16 packing); the
            # 2-src scalar_tensor_tensor variant would be 1x, so two ts ops
            # beat one stt by ~2x.
            acc = small.tile([_P, _NCHUNK], dt.float32)
            neg_t = small.tile([_P, 1], dt.float32)

            def final_chunk(j):
                if "final" not in _STAGES:
                    return
                w, off = _CHUNK_W[j], _CHUNK_OFF[j]
                rl = tmp_pool.tile([_P, w], dt.bfloat16, tag="rl")
                vj = V[:, off:off + w]
                if _FIN_ENG[j] == "a":
                    nc.scalar.activation(
                        rl[:], vj, Act.Relu,
                        bias=neg_t[:], accum_out=acc[:, j:j + 1],
                    )
                else:
                    eng = nc.gpsimd if _FIN_ENG[j] == "g" else nc.vector
                    # rl = max(v - t, 0)
                    eng.tensor_scalar(
                        rl[:], vj, t_bc[:], 0.0, Alu.subtract, Alu.max
                    )
                    # accum_out = sum(rl)   (op0 max(.,0) is a no-op on rl)
                    eng.tensor_scalar(
                        rl[:], rl[:], 0.0, None, Alu.max, Alu.add,
                        accum_out=acc[:, j:j + 1],
                    )

            # --- main pipeline ---
            # chunks 0/1 first, then the Newton round (counts on chunk 1),
            # then finals interleaved with remaining production so the only
            # work after the last ln is one small sub + final.
            # reps > 1 serially repeats the body (device-time measurement).
            round_chunks = max(r[0] for r in _ROUNDS) + 1 if _ROUNDS else 0

            def rep_body():
                nc.vector.memset(t_bc[:], _T0)
                nc.vector.memset(t8[:], _T0)
                nc.vector.memset(acc[:], 0.0)
                for j in range(round_chunks):
                    produce_chunk(j)
                if "newton" in _STAGES:
                    for ridx, (chunk, width, h, clamp) in enumerate(_ROUNDS):
                        newton_round(ridx, chunk, width, h, clamp)
                nc.vector.tensor_scalar_mul(neg_t[:], t_bc[:], -1.0)
                for j in range(round_chunks):
                    final_chunk(j)
                for j in range(round_chunks, _NCHUNK):
                    produce_chunk(j)
                    final_chunk(j)

                out_t = small.tile([_P, 2], dt.float32)
                nc.vector.tensor_reduce(
                    out_t[:, 0:1], acc[:], mybir.AxisListType.X, Alu.add
                )
                nc.vector.tensor_copy(out_t[:, 1:2], t_bc[:])
                nc.sync.dma_start(out_ext[:], out_t[:])

            if loop is None:
                for _rep in range(reps):
                    rep_body()
            else:
                with tc.For_i(0, loop, 1):
                    for _rep in range(reps):
                        rep_body()

    # Steer the ACT table chooser to the one set holding BOTH Exp and Ln so
    # it loads a single table instead of thrashing exp<->ln sets per chunk.
    # Only the chooser's view is filtered; list order (= act_func_set_id)
    # is preserved, and the runtime set genuinely contains both functions.
    from concourse import bacc as _bacc_mod
    _orig_tables = _bacc_mod.get_activation_tables

    def _steered_tables(arch):
        tabs = dict(_orig_tables(arch))
        # pin every function this kernel uses to the one set with Exp+Ln
        used = {Act.Exp, Act.Ln, Act.Sign, Act.Relu}
        combined = [n for n, fns in tabs.items() if {Act.Exp, Act.Ln} <= fns]
        if combined:
            keep = combined[0]
            tabs = {
                n: (fns if n == keep else (fns - used))
                for n, fns in tabs.items()
            }
        return tabs

    _bacc_mod.get_activation_tables = _steered_tables
    try:
        nc.compile()
    finally:
        _bacc_mod.get_activation_tables = _orig_tables
    return nc


def _shard_inputs(pred_logits, gts):
    x = np.ascontiguousarray(pred_logits, dtype=np.float32).reshape(_B, _HW)
    y = np.ascontiguousarray(gts, dtype=np.float32).reshape(_B, _HW)
    xb = x.astype(_BF16)
    yb = y.astype(_BF16)
    in_maps = []
    for c in range(_NCORES):
        sl = slice(c * _RPC, (c + 1) * _RPC)
        xs = xb[sl].reshape(_P, _FREE)
        ys = yb[sl].reshape(_P, _FREE)
        # interleave x/y per (variable-width) chunk: [... x_w | y_w ...]
        blocks = []
        for w, off in zip(_CHUNK_W, _CHUNK_OFF):
            blocks.append(xs[:, off:off + w])
            blocks.append(ys[:, off:off + w])
        xy = np.concatenate(blocks, axis=1)
        in_maps.append({"xy": np.ascontiguousarray(xy)})
    return in_maps


def _combine(results):
    total = 0.0
    for c in range(_NCORES):
        out = np.asarray(results[c]["out"], dtype=np.float64)  # [128, 2]
        total += out[:, 0].sum()
        total += _K * out[::_PPR, 1].sum()
    return np.float32(total / (_B * _K))


def kernel(pred_logits, gts):
    from concourse.bass_utils import run_bass_kernel_spmd

    global _cached_nc
    if _cached_nc is None:
        _cached_nc = build_bass()
    in_maps = _shard_inputs(pred_logits, gts)
    res = run_bass_kernel_spmd(_cached_nc, in_maps, list(range(_NCORES)))
    return _combine(res.results)

